# revision 20
# baseline (speedup 1.0000x reference)
"""Trainium2 Bass kernel for nn_Attention_43946105373274.

Causal multi-head attention with rotary embeddings applied to q, k and v.
B=2, N=2048, DIM=1024, H=16, DH=64, f32.

Sharding: 8 cores = (2 batches) x (4 head-groups of 4 heads).
Each core computes qkv projection for its heads (w_qkv column-shard),
full causal attention for its heads, and a partial output projection
(w_out row-shard).  The host sums the 4 partials per batch (the
"all-reduce" of the output projection) — full inputs in, full output out.
"""

import sys
import numpy as np

if "/opt/trn_rl_repo" not in sys.path:
    sys.path.insert(0, "/opt/trn_rl_repo")

B, N, DIM, H, DH = 2, 2048, 1024, 16, 64
HPC = 4                     # heads per core
NCORES = 8
SCALE = DH ** -0.5
NT = N // 128               # 16 row tiles
KB = DIM // 128             # 8 contraction blocks
JT = 3 * HPC * DH // 128    # 6 qkv j-tiles (q01,q23,k01,k23,v01,v23)
CW = 512                    # i-chunk width
NCH = N // CW               # 4 chunks

_CACHE = {}


def _build_program():
    import concourse.bass as bass  # noqa: F401
    import concourse.mybir as mybir
    import concourse.tile as tile
    from concourse import bacc

    F32 = mybir.dt.float32
    F32R = mybir.dt.float32r
    BF16 = mybir.dt.bfloat16
    AF = mybir.ActivationFunctionType
    OP = mybir.AluOpType

    nc = bacc.Bacc("TRN2", target_bir_lowering=False, debug=False,
                   num_devices=NCORES)

    xb = nc.dram_tensor("xb", [N, DIM], BF16, kind="ExternalInput")
    wqkv = nc.dram_tensor("wqkv", [DIM, 3 * HPC * DH], BF16, kind="ExternalInput")
    wout = nc.dram_tensor("wout", [HPC * DH, DIM], BF16, kind="ExternalInput")
    bvec = nc.dram_tensor("bvec", [1, DIM], BF16, kind="ExternalInput")
    freqs = nc.dram_tensor("freqs", [N, DH], F32, kind="ExternalInput")
    identD = nc.dram_tensor("identD", [128, 128], F32R, kind="ExternalInput")
    rmatD = nc.dram_tensor("rmatD", [128, 128], BF16, kind="ExternalInput")
    identB = nc.dram_tensor("identB", [128, 128], BF16, kind="ExternalInput")
    outD = nc.dram_tensor("out", [N, DIM], F32, kind="ExternalOutput")

    MAGIC = 12582912.0          # 1.5 * 2**23: float32 round-to-nearest trick
    TWO_PI = float(2 * np.pi)

    with tile.TileContext(nc) as tc:
        with tc.tile_pool(name="pc", bufs=1) as pc, \
             tc.tile_pool(name="pw", bufs=KB) as pw, \
             tc.tile_pool(name="pwo", bufs=2) as pwo, \
             tc.tile_pool(name="pxT", bufs=8) as pxT, \
             tc.tile_pool(name="pqk", bufs=4) as pqk, \
             tc.tile_pool(name="pv", bufs=4) as pv, \
             tc.tile_pool(name="pst", bufs=3) as pst, \
             tc.tile_pool(name="ppt", bufs=4) as ppt, \
             tc.tile_pool(name="poT", bufs=2) as poT, \
             tc.tile_pool(name="pnm", bufs=2) as pnm, \
             tc.tile_pool(name="pout", bufs=2) as pout, \
             tc.tile_pool(name="psA", bufs=2, space="PSUM") as psA, \
             tc.tile_pool(name="psT", bufs=2, space="PSUM") as psT, \
             tc.tile_pool(name="psC", bufs=1, space="PSUM") as psC:

            # ---------------- phase 0: constants & freqs prep ----------------
            ident = pc.tile([128, 128], F32R, tag="ident")
            nc.sync.dma_start(ident[:], identD[:])
            rmat = pc.tile([128, 128], BF16, tag="rmat")
            nc.sync.dma_start(rmat[:], rmatD[:])
            ones_f = pc.tile([128, 128], F32, tag="ones_f")
            nc.vector.memset(ones_f[:], 1.0)
            ones_r = pc.tile([1, 128], F32R, tag="ones_r")
            nc.vector.tensor_copy(ones_r[:], ones_f[0:1, :])
            identb = pc.tile([128, 128], BF16, tag="identb")
            nc.sync.dma_start(identb[:], identB[:])
            ones_b = pc.tile([1, 128], BF16, tag="ones_b")
            nc.vector.tensor_copy(ones_b[:], ones_f[0:1, :])
            btile_b = pc.tile([1, DIM], BF16, tag="btile_b")
            nc.sync.dma_start(btile_b[:], bvec[:])

            # weights
            w_sb = []
            for kb in range(KB):
                wt = pw.tile([128, 3 * HPC * DH], BF16, tag="w")
                nc.sync.dma_start(wt[:], wqkv[kb * 128:(kb + 1) * 128, :])
                w_sb.append(wt)
            wo_sb = []
            for cb in range(2):
                wt = pwo.tile([128, DIM], BF16, tag="wo")
                nc.sync.dma_start(wt[:], wout[cb * 128:(cb + 1) * 128, :])
                wo_sb.append(wt)

            # freqs -> cos/sin, transposed to [64, 2048], duplicated across
            # both 64-row halves (two heads share one pair-tile).
            ftile = pc.tile([128, NT, DH], F32, tag="ftile")
            fview = freqs[:].rearrange("(t p) d -> p t d", p=128)
            nc.sync.dma_start(ftile[:], fview)
            cosT2 = pc.tile([128, N], F32, tag="cosT2")
            sinT2 = pc.tile([128, N], F32, tag="sinT2")
            for which in range(2):  # 0: sin, 1: cos
                y = pout.tile([128, NT * DH], F32, tag="osb", bufs=3)
                if which == 1:
                    nc.vector.tensor_scalar_add(y[:], ftile[:].rearrange("p t d -> p (t d)"),
                                                float(np.pi / 2))
                else:
                    nc.vector.tensor_copy(y[:], ftile[:].rearrange("p t d -> p (t d)"))
                k = pout.tile([128, NT * DH], F32, tag="osb", bufs=3)
                nc.vector.tensor_scalar_mul(k[:], y[:], float(1.0 / TWO_PI))
                nc.vector.tensor_scalar_add(k[:], k[:], MAGIC)
                nc.vector.tensor_scalar_sub(k[:], k[:], MAGIC)
                xr = pout.tile([128, NT * DH], F32, tag="osb", bufs=3)
                nc.vector.scalar_tensor_tensor(xr[:], k[:], -TWO_PI, y[:],
                                               op0=OP.mult, op1=OP.add)
                trig = pout.tile([128, NT, DH], F32R, tag="trig", bufs=1)
                nc.scalar.activation(trig[:].rearrange("p t d -> p (t d)"), xr[:], AF.Sin)
                dst = cosT2 if which == 1 else sinT2
                for tg in range(NT // 4):
                    fps = psC.tile([64, 512], F32R, tag="ps512f", bufs=2)
                    for q in range(4):
                        t = tg * 4 + q
                        nc.tensor.transpose(fps[:, q * 128:(q + 1) * 128],
                                            trig[:, t, :], ident[:])
                    nc.vector.tensor_copy(dst[0:64, tg * 512:(tg + 1) * 512], fps[:])
                    nc.vector.tensor_copy(dst[64:128, tg * 512:(tg + 1) * 512], fps[:])

            # persistent tensors
            qT = [pqk.tile([128, N], BF16, tag="qk", name=f"qT{i}") for i in range(2)]
            kT = [pqk.tile([128, N], BF16, tag="qk", name=f"kT{i}") for i in range(2)]
            # V tiles: [128, 65] per (head, row-tile); col 64 = ones
            vt = [pv.tile([128, NT * (DH + 1)], BF16, tag="v", name=f"vt{h}", bufs=4)
                  for h in range(HPC)]
            for h in range(HPC):
                vv = vt[h][:].rearrange("p (t c) -> p t c", c=DH + 1)
                nc.vector.tensor_copy(vv[:, :, DH:DH + 1],
                                      ones_f[:, 0:NT].unsqueeze(2))
            oT = [poT.tile([128, N], BF16, tag="oT", name=f"oT{i}") for i in range(2)]

            # ---------------- phase 1: x^T (bf16), qkv projection, rotary ----
            for cp in range(2):            # 1024-column n blocks
                xT8 = [pxT.tile([128, 1024], BF16, tag="xT", bufs=12,
                                name=f"xT_{cp}_{kb}") for kb in range(KB)]
                n0 = cp * 1024
                for kb in range(KB):
                    nc.sync.dma_start_transpose(
                        xT8[kb][:], xb[n0:n0 + 1024, kb * 128:(kb + 1) * 128])

                for jt in range(JT):
                    qps = psA.tile([128, 1024], F32, tag="psA")
                    for kb in range(KB):
                        for mh in range(2):
                            nc.tensor.matmul(
                                qps[:, mh * 512:(mh + 1) * 512],
                                w_sb[kb][:, jt * 128:(jt + 1) * 128],
                                xT8[kb][:, mh * 512:(mh + 1) * 512],
                                start=(kb == 0), stop=(kb == KB - 1))
                    for half in range(2):
                        c = cp * 2 + half
                        ph = qps[:, half * 512:(half + 1) * 512]
                        csl = cosT2[:, c * CW:(c + 1) * CW]
                        ssl = sinT2[:, c * CW:(c + 1) * CW]
                        t_sb = pst.tile([128, CW], BF16, tag="t_sb", bufs=2)
                        nc.scalar.activation(t_sb[:], ph, AF.Copy)
                        rps = psC.tile([128, CW], F32, tag="ps512f", bufs=2)
                        nc.tensor.matmul(rps[:], rmat[:], t_sb[:], start=True, stop=True)
                        tmp = pst.tile([128, CW], F32, tag="tmp", bufs=2)
                        nc.vector.tensor_mul(tmp[:], ph, csl)
                        rs = pst.tile([128, CW], F32, tag="rs", bufs=2)
                        nc.vector.tensor_mul(rs[:], rps[:], ssl)
                        if jt < 4:  # q or k -> straight into qT/kT
                            dst = qT[jt] if jt < 2 else kT[jt - 2]
                            nc.vector.tensor_add(dst[:, c * CW:(c + 1) * CW],
                                                 tmp[:], rs[:])
                        else:       # v -> rotate then transpose into V tiles
                            v_sb = pst.tile([128, CW], BF16, tag="v_sb", bufs=2)
                            nc.vector.tensor_add(v_sb[:], tmp[:], rs[:])
                            pair = jt - 4
                            vps = psT.tile([128, CW], BF16, tag="pstr", bufs=1)
                            for rt in range(4):
                                nc.tensor.transpose(
                                    vps[:, rt * 128:(rt + 1) * 128],
                                    v_sb[:, rt * 128:(rt + 1) * 128],
                                    identb[:])
                            vpsv = vps[:].rearrange("p (t hh d) -> p t hh d", t=4, hh=2)
                            for hh in range(2):
                                h = pair * 2 + hh
                                dstv = vt[h][:].rearrange("p (t c) -> p t c", c=DH + 1)[
                                    :, c * 4:(c + 1) * 4, 0:DH]
                                nc.vector.tensor_copy(dstv, vpsv[:, :, hh, :])

            # ---------------- phase 2: attention per head --------------------
            for h in range(HPC):
                pair, hh = h // 2, h % 2
                qh = qT[pair][hh * 64:(hh + 1) * 64, :]
                kh = kT[pair][hh * 64:(hh + 1) * 64, :]
                for c in range(NCH):
                    nj = 4 * c + 4          # j-blocks needed (causal)
                    av = psC.tile([DH + 1, CW], F32, tag="ps512f", bufs=2)
                    for grp in range(nj // 2):
                        j0 = grp * 2
                        sps = psA.tile([128, 1024], F32, tag="psA")
                        for g in range(2):
                            j = j0 + g
                            nc.tensor.matmul(
                                sps[:, g * 512:(g + 1) * 512],
                                kh[:, j * 128:(j + 1) * 128],
                                qh[:, c * CW:(c + 1) * CW],
                                start=True, stop=True)
                        pt = ppt.tile([128, 1024], BF16, tag="pt")
                        nc.scalar.activation(pt[:], sps[:], AF.Exp, scale=SCALE)
                        if j0 + 1 >= 4 * c:  # group touches the diagonal
                            ptv = pt[:].rearrange("p (g i) -> p g i", g=2)
                            nc.gpsimd.affine_select(
                                out=ptv, in_=ptv,
                                compare_op=OP.is_ge, fill=0.0,
                                base=c * CW - j0 * 128,
                                pattern=[[-128, 2], [1, CW]],
                                channel_multiplier=-1)
                        for g in range(2):
                            j = j0 + g
                            nc.tensor.matmul(av[:],
                                             vt[h][:, j * (DH + 1):(j + 1) * (DH + 1)],
                                             pt[:, g * 512:(g + 1) * 512],
                                             start=(j == 0), stop=(j == nj - 1))
                    # normalization for this chunk
                    s_r = pnm.tile([1, CW], F32R, tag="s_r", bufs=2)
                    nc.vector.tensor_copy(s_r[:], av[DH:DH + 1, :])
                    rbp = psT.tile([64, CW], F32, tag="rbp", bufs=1)
                    nc.tensor.matmul(rbp[:], ones_r[0:1, 0:64], s_r[:],
                                     start=True, stop=True)
                    rb = pnm.tile([64, CW], F32, tag="rb")
                    nc.vector.reciprocal_approx_fast(rb[:], rbp[:])
                    osl = oT[pair][hh * 64:(hh + 1) * 64, c * CW:(c + 1) * CW]
                    nc.vector.tensor_mul(osl, av[0:DH, :], rb[:])

            # ---------------- phase 3: output projection ---------------------
            for nt_i in range(NT):
                prj = psA.tile([128, DIM], F32, tag="psA")
                for mh in range(2):
                    for cb in range(2):
                        nc.tensor.matmul(
                            prj[:, mh * 512:(mh + 1) * 512],
                            oT[cb][:, nt_i * 128:(nt_i + 1) * 128],
                            wo_sb[cb][:, mh * 512:(mh + 1) * 512],
                            start=(cb == 0), stop=False)
                    nc.tensor.matmul(
                        prj[:, mh * 512:(mh + 1) * 512],
                        ones_b[0:1, :], btile_b[0:1, mh * 512:(mh + 1) * 512],
                        start=False, stop=True)
                ot = pout.tile([128, DIM], F32, tag="osb", bufs=3)
                nc.vector.tensor_copy(ot[:], prj[:])
                nc.sync.dma_start(outD[nt_i * 128:(nt_i + 1) * 128, :], ot[:])

    nc.compile()
    return nc


def _get_program():
    if "nc" not in _CACHE:
        _CACHE["nc"] = _build_program()
    return _CACHE["nc"]


def _rot_lhsT():
    """lhsT for rot_half: out = lhsT.T @ tT = R @ tT, interleaved pairs."""
    R64 = np.zeros((64, 64), np.float32)
    for i in range(32):
        R64[2 * i, 2 * i + 1] = -1.0
        R64[2 * i + 1, 2 * i] = 1.0
    R = np.zeros((128, 128), np.float32)
    R[0:64, 0:64] = R64
    R[64:128, 64:128] = R64
    return np.ascontiguousarray(R.T)


def make_in_maps(x, rotary_pos_emb, w_qkv, w_out, b_out):
    x = np.asarray(x, np.float32)
    rotary_pos_emb = np.asarray(rotary_pos_emb, np.float32)
    w_qkv = np.asarray(w_qkv, np.float32)
    w_out = np.asarray(w_out, np.float32)
    b_out = np.asarray(b_out, np.float32)

    import ml_dtypes
    bf16 = ml_dtypes.bfloat16
    ident = np.eye(128, dtype=np.float32)
    identb = np.eye(128).astype(bf16)
    rmatT = _rot_lhsT()
    zeros_b = np.zeros((1, DIM), bf16)

    in_maps = []
    for c in range(NCORES):
        b = c // 4
        heads = [4 * (c % 4) + i for i in range(HPC)]
        # w_qkv column shard in j-tile order: q01,q23,k01,k23,v01,v23
        cols = []
        for t in range(3):            # q, k, v
            for h in heads:
                cols.append(w_qkv[:, t * H * DH + h * DH: t * H * DH + (h + 1) * DH])
        w_s = np.ascontiguousarray(np.concatenate(cols, axis=1))
        w_o = np.ascontiguousarray(
            np.concatenate([w_out[h * DH:(h + 1) * DH, :] for h in heads], axis=0))
        in_maps.append({
            "xb": np.ascontiguousarray(x[b]).astype(bf16),
            "wqkv": w_s.astype(bf16),
            "wout": w_o.astype(bf16),
            "bvec": (b_out[None, :] if c % 4 == 0 else zeros_b).astype(bf16),
            "freqs": rotary_pos_emb,
            "identD": ident,
            "rmatD": rmatT.astype(bf16),
            "identB": identb,
        })
    return in_maps


def kernel(x, rotary_pos_emb, w_qkv, w_out, b_out):
    from concourse.bass_utils import run_bass_kernel_spmd

    nc = _get_program()
    in_maps = make_in_maps(x, rotary_pos_emb, w_qkv, w_out, b_out)
    res = run_bass_kernel_spmd(nc, in_maps, list(range(NCORES))).results

    out = np.zeros((B, N, DIM), np.float32)
    for c in range(NCORES):
        out[c // 4] += res[c]["out"]
    return out


# revision 21
# speedup vs baseline: 1.2561x; 1.2561x over previous
"""Trainium2 Bass kernel for nn_Attention_43946105373274.

Causal multi-head attention with rotary embeddings applied to q, k and v.
B=2, N=2048, DIM=1024, H=16, DH=64, f32.

Sharding: 8 cores = (2 batches) x (4 head-groups of 4 heads).
Each core computes qkv projection for its heads (w_qkv column-shard),
full causal attention for its heads, and a partial output projection
(w_out row-shard).  The host sums the 4 partials per batch (the
"all-reduce" of the output projection) — full inputs in, full output out.
"""

import sys
import numpy as np

if "/opt/trn_rl_repo" not in sys.path:
    sys.path.insert(0, "/opt/trn_rl_repo")

B, N, DIM, H, DH = 2, 2048, 1024, 16, 64
HPC = 4                     # heads per core
NCORES = 8
SCALE = DH ** -0.5
NT = N // 128               # 16 row tiles
KB = DIM // 128             # 8 contraction blocks
JT = 3 * HPC * DH // 128    # 6 qkv j-tiles (q01,q23,k01,k23,v01,v23)
CW = 512                    # i-chunk width
NCH = N // CW               # 4 chunks

_CACHE = {}


def _build_program():
    import concourse.bass as bass  # noqa: F401
    import concourse.mybir as mybir
    import concourse.tile as tile
    from concourse import bacc

    F32 = mybir.dt.float32
    F32R = mybir.dt.float32r
    BF16 = mybir.dt.bfloat16
    AF = mybir.ActivationFunctionType
    OP = mybir.AluOpType

    nc = bacc.Bacc("TRN2", target_bir_lowering=False, debug=False,
                   num_devices=NCORES)

    xb = nc.dram_tensor("xb", [N, DIM], BF16, kind="ExternalInput")
    wqkv = nc.dram_tensor("wqkv", [DIM, 3 * HPC * DH], BF16, kind="ExternalInput")
    wout = nc.dram_tensor("wout", [HPC * DH, DIM], BF16, kind="ExternalInput")
    bvec = nc.dram_tensor("bvec", [1, DIM], BF16, kind="ExternalInput")
    freqs = nc.dram_tensor("freqs", [N, DH], F32, kind="ExternalInput")
    identD = nc.dram_tensor("identD", [128, 128], F32R, kind="ExternalInput")
    rmatD = nc.dram_tensor("rmatD", [128, 128], BF16, kind="ExternalInput")
    identB = nc.dram_tensor("identB", [128, 128], BF16, kind="ExternalInput")
    outD = nc.dram_tensor("out", [N, DIM], F32, kind="ExternalOutput")

    MAGIC = 12582912.0          # 1.5 * 2**23: float32 round-to-nearest trick
    TWO_PI = float(2 * np.pi)

    with tile.TileContext(nc) as tc:
        with tc.tile_pool(name="pc", bufs=1) as pc, \
             tc.tile_pool(name="pw", bufs=KB) as pw, \
             tc.tile_pool(name="pwo", bufs=2) as pwo, \
             tc.tile_pool(name="pxT", bufs=8) as pxT, \
             tc.tile_pool(name="pqk", bufs=4) as pqk, \
             tc.tile_pool(name="pv", bufs=4) as pv, \
             tc.tile_pool(name="pst", bufs=3) as pst, \
             tc.tile_pool(name="ppt", bufs=4) as ppt, \
             tc.tile_pool(name="poT", bufs=2) as poT, \
             tc.tile_pool(name="pnm", bufs=2) as pnm, \
             tc.tile_pool(name="pout", bufs=2) as pout, \
             tc.tile_pool(name="psA", bufs=2, space="PSUM") as psA, \
             tc.tile_pool(name="psT", bufs=2, space="PSUM") as psT, \
             tc.tile_pool(name="psC", bufs=1, space="PSUM") as psC:

            # ---------------- phase 0: constants & freqs prep ----------------
            ident = pc.tile([128, 128], F32R, tag="ident")
            nc.sync.dma_start(ident[:], identD[:])
            rmat = pc.tile([128, 128], BF16, tag="rmat")
            nc.sync.dma_start(rmat[:], rmatD[:])
            ones_f = pc.tile([128, 128], F32, tag="ones_f")
            nc.vector.memset(ones_f[:], 1.0)
            ones_r = pc.tile([1, 128], F32R, tag="ones_r")
            nc.vector.tensor_copy(ones_r[:], ones_f[0:1, :])
            identb = pc.tile([128, 128], BF16, tag="identb")
            nc.sync.dma_start(identb[:], identB[:])
            ones_b = pc.tile([1, 128], BF16, tag="ones_b")
            nc.vector.tensor_copy(ones_b[:], ones_f[0:1, :])
            btile_b = pc.tile([1, DIM], BF16, tag="btile_b")
            nc.sync.dma_start(btile_b[:], bvec[:])

            # weights
            w_sb = []
            for kb in range(KB):
                wt = pw.tile([128, 3 * HPC * DH], BF16, tag="w")
                nc.sync.dma_start(wt[:], wqkv[kb * 128:(kb + 1) * 128, :])
                w_sb.append(wt)
            wo_sb = []
            for cb in range(2):
                wt = pwo.tile([128, DIM], BF16, tag="wo")
                nc.sync.dma_start(wt[:], wout[cb * 128:(cb + 1) * 128, :])
                wo_sb.append(wt)

            # freqs -> cos/sin, transposed to [64, 2048], duplicated across
            # both 64-row halves (two heads share one pair-tile).
            ftile = pc.tile([128, NT, DH], F32, tag="ftile")
            fview = freqs[:].rearrange("(t p) d -> p t d", p=128)
            nc.sync.dma_start(ftile[:], fview)
            cosT2 = pc.tile([128, N], F32, tag="cosT2")
            sinT2 = pc.tile([128, N], F32, tag="sinT2")
            for which in range(2):  # 0: sin, 1: cos
                y = pout.tile([128, NT * DH], F32, tag="osb", bufs=3)
                if which == 1:
                    nc.vector.tensor_scalar_add(y[:], ftile[:].rearrange("p t d -> p (t d)"),
                                                float(np.pi / 2))
                else:
                    nc.vector.tensor_copy(y[:], ftile[:].rearrange("p t d -> p (t d)"))
                k = pout.tile([128, NT * DH], F32, tag="osb", bufs=3)
                nc.vector.tensor_scalar_mul(k[:], y[:], float(1.0 / TWO_PI))
                nc.vector.tensor_scalar_add(k[:], k[:], MAGIC)
                nc.vector.tensor_scalar_sub(k[:], k[:], MAGIC)
                xr = pout.tile([128, NT * DH], F32, tag="osb", bufs=3)
                nc.vector.scalar_tensor_tensor(xr[:], k[:], -TWO_PI, y[:],
                                               op0=OP.mult, op1=OP.add)
                trig = pout.tile([128, NT, DH], F32R, tag="trig", bufs=1)
                nc.scalar.activation(trig[:].rearrange("p t d -> p (t d)"), xr[:], AF.Sin)
                dst = cosT2 if which == 1 else sinT2
                for tg in range(NT // 4):
                    fps = psC.tile([64, 512], F32R, tag="ps512f", bufs=2)
                    for q in range(4):
                        t = tg * 4 + q
                        nc.tensor.transpose(fps[:, q * 128:(q + 1) * 128],
                                            trig[:, t, :], ident[:])
                    nc.vector.tensor_copy(dst[0:64, tg * 512:(tg + 1) * 512], fps[:])
                    nc.vector.tensor_copy(dst[64:128, tg * 512:(tg + 1) * 512], fps[:])

            # persistent tensors
            qT = [pqk.tile([128, N], BF16, tag="qk", name=f"qT{i}") for i in range(2)]
            kT = [pqk.tile([128, N], BF16, tag="qk", name=f"kT{i}") for i in range(2)]
            # V tiles: [128, 65] per (head, row-tile); col 64 = ones
            vt = [pv.tile([128, NT * (DH + 1)], BF16, tag="v", name=f"vt{h}", bufs=4)
                  for h in range(HPC)]
            for h in range(HPC):
                vv = vt[h][:].rearrange("p (t c) -> p t c", c=DH + 1)
                nc.vector.tensor_copy(vv[:, :, DH:DH + 1],
                                      ones_f[:, 0:NT].unsqueeze(2))
            oT = [poT.tile([128, N], BF16, tag="oT", name=f"oT{i}") for i in range(2)]

            # ---------------- phase 1: x^T (bf16), qkv projection, rotary ----
            for cp in range(2):            # 1024-column n blocks
                xT8 = [pxT.tile([128, 1024], BF16, tag="xT", bufs=12,
                                name=f"xT_{cp}_{kb}") for kb in range(KB)]
                n0 = cp * 1024
                for kb in range(KB):
                    nc.sync.dma_start_transpose(
                        xT8[kb][:], xb[n0:n0 + 1024, kb * 128:(kb + 1) * 128])

                for jt in range(JT):
                    qps = psA.tile([128, 1024], F32, tag="psA")
                    for kb in range(KB):
                        for mh in range(2):
                            nc.tensor.matmul(
                                qps[:, mh * 512:(mh + 1) * 512],
                                w_sb[kb][:, jt * 128:(jt + 1) * 128],
                                xT8[kb][:, mh * 512:(mh + 1) * 512],
                                start=(kb == 0), stop=(kb == KB - 1))
                    for half in range(2):
                        c = cp * 2 + half
                        ph = qps[:, half * 512:(half + 1) * 512]
                        csl = cosT2[:, c * CW:(c + 1) * CW]
                        ssl = sinT2[:, c * CW:(c + 1) * CW]
                        t_sb = pst.tile([128, CW], BF16, tag="t_sb", bufs=2)
                        nc.scalar.activation(t_sb[:], ph, AF.Copy)
                        rps = psC.tile([128, CW], F32, tag="ps512f", bufs=2)
                        nc.tensor.matmul(rps[:], rmat[:], t_sb[:], start=True, stop=True)
                        tmp = pst.tile([128, CW], F32, tag="tmp", bufs=2)
                        nc.vector.tensor_mul(tmp[:], ph, csl)
                        rs = pst.tile([128, CW], F32, tag="rs", bufs=2)
                        nc.vector.tensor_mul(rs[:], rps[:], ssl)
                        if jt < 4:  # q or k -> straight into qT/kT
                            dst = qT[jt] if jt < 2 else kT[jt - 2]
                            nc.vector.tensor_add(dst[:, c * CW:(c + 1) * CW],
                                                 tmp[:], rs[:])
                        else:       # v -> rotate then transpose into V tiles
                            v_sb = pst.tile([128, CW], BF16, tag="v_sb", bufs=2)
                            nc.vector.tensor_add(v_sb[:], tmp[:], rs[:])
                            pair = jt - 4
                            vps = psT.tile([128, CW], BF16, tag="pstr", bufs=2)
                            for rt in range(4):
                                nc.tensor.transpose(
                                    vps[:, rt * 128:(rt + 1) * 128],
                                    v_sb[:, rt * 128:(rt + 1) * 128],
                                    identb[:])
                            vpsv = vps[:].rearrange("p (t hh d) -> p t hh d", t=4, hh=2)
                            for hh in range(2):
                                h = pair * 2 + hh
                                dstv = vt[h][:].rearrange("p (t c) -> p t c", c=DH + 1)[
                                    :, c * 4:(c + 1) * 4, 0:DH]
                                nc.vector.tensor_copy(dstv, vpsv[:, :, hh, :])

            # ---------------- phase 2: attention per head --------------------
            for h in range(HPC):
                pair, hh = h // 2, h % 2
                qh = qT[pair][hh * 64:(hh + 1) * 64, :]
                kh = kT[pair][hh * 64:(hh + 1) * 64, :]
                for c in range(NCH):
                    nj = 4 * c + 4          # j-blocks needed (causal)
                    av = psC.tile([DH + 1, CW], F32, tag="ps512f", bufs=2)
                    for grp in range(nj // 2):
                        j0 = grp * 2
                        sps = psA.tile([128, 1024], F32, tag="psA")
                        for g in range(2):
                            j = j0 + g
                            nc.tensor.matmul(
                                sps[:, g * 512:(g + 1) * 512],
                                kh[:, j * 128:(j + 1) * 128],
                                qh[:, c * CW:(c + 1) * CW],
                                start=True, stop=True)
                        pt = ppt.tile([128, 1024], BF16, tag="pt")
                        nc.scalar.activation(pt[:], sps[:], AF.Exp, scale=SCALE)
                        if j0 + 1 >= 4 * c:  # group touches the diagonal
                            ptv = pt[:].rearrange("p (g i) -> p g i", g=2)
                            nc.gpsimd.affine_select(
                                out=ptv, in_=ptv,
                                compare_op=OP.is_ge, fill=0.0,
                                base=c * CW - j0 * 128,
                                pattern=[[-128, 2], [1, CW]],
                                channel_multiplier=-1)
                        for g in range(2):
                            j = j0 + g
                            nc.tensor.matmul(av[:],
                                             vt[h][:, j * (DH + 1):(j + 1) * (DH + 1)],
                                             pt[:, g * 512:(g + 1) * 512],
                                             start=(j == 0), stop=(j == nj - 1))
                    # normalization for this chunk
                    s_r = pnm.tile([1, CW], F32R, tag="s_r", bufs=2)
                    nc.vector.tensor_copy(s_r[:], av[DH:DH + 1, :])
                    rbp = psC.tile([64, CW], F32, tag="ps512f", bufs=2)
                    nc.tensor.matmul(rbp[:], ones_r[0:1, 0:64], s_r[:],
                                     start=True, stop=True)
                    rb = pnm.tile([64, CW], F32, tag="rb")
                    nc.vector.reciprocal_approx_fast(rb[:], rbp[:])
                    osl = oT[pair][hh * 64:(hh + 1) * 64, c * CW:(c + 1) * CW]
                    nc.vector.tensor_mul(osl, av[0:DH, :], rb[:])

            # ---------------- phase 3: output projection ---------------------
            for nt_i in range(NT):
                prj = psA.tile([128, DIM], F32, tag="psA")
                for mh in range(2):
                    for cb in range(2):
                        nc.tensor.matmul(
                            prj[:, mh * 512:(mh + 1) * 512],
                            oT[cb][:, nt_i * 128:(nt_i + 1) * 128],
                            wo_sb[cb][:, mh * 512:(mh + 1) * 512],
                            start=(cb == 0), stop=False)
                    nc.tensor.matmul(
                        prj[:, mh * 512:(mh + 1) * 512],
                        ones_b[0:1, :], btile_b[0:1, mh * 512:(mh + 1) * 512],
                        start=False, stop=True)
                ot = pout.tile([128, DIM], F32, tag="osb", bufs=3)
                nc.vector.tensor_copy(ot[:], prj[:])
                nc.sync.dma_start(outD[nt_i * 128:(nt_i + 1) * 128, :], ot[:])

    nc.compile()
    return nc


def _get_program():
    if "nc" not in _CACHE:
        _CACHE["nc"] = _build_program()
    return _CACHE["nc"]


def _rot_lhsT():
    """lhsT for rot_half: out = lhsT.T @ tT = R @ tT, interleaved pairs."""
    R64 = np.zeros((64, 64), np.float32)
    for i in range(32):
        R64[2 * i, 2 * i + 1] = -1.0
        R64[2 * i + 1, 2 * i] = 1.0
    R = np.zeros((128, 128), np.float32)
    R[0:64, 0:64] = R64
    R[64:128, 64:128] = R64
    return np.ascontiguousarray(R.T)


def make_in_maps(x, rotary_pos_emb, w_qkv, w_out, b_out):
    x = np.asarray(x, np.float32)
    rotary_pos_emb = np.asarray(rotary_pos_emb, np.float32)
    w_qkv = np.asarray(w_qkv, np.float32)
    w_out = np.asarray(w_out, np.float32)
    b_out = np.asarray(b_out, np.float32)

    import ml_dtypes
    bf16 = ml_dtypes.bfloat16
    ident = np.eye(128, dtype=np.float32)
    identb = np.eye(128).astype(bf16)
    rmatT = _rot_lhsT()
    zeros_b = np.zeros((1, DIM), bf16)

    in_maps = []
    for c in range(NCORES):
        b = c // 4
        heads = [4 * (c % 4) + i for i in range(HPC)]
        # w_qkv column shard in j-tile order: q01,q23,k01,k23,v01,v23
        cols = []
        for t in range(3):            # q, k, v
            for h in heads:
                cols.append(w_qkv[:, t * H * DH + h * DH: t * H * DH + (h + 1) * DH])
        w_s = np.ascontiguousarray(np.concatenate(cols, axis=1))
        w_o = np.ascontiguousarray(
            np.concatenate([w_out[h * DH:(h + 1) * DH, :] for h in heads], axis=0))
        in_maps.append({
            "xb": np.ascontiguousarray(x[b]).astype(bf16),
            "wqkv": w_s.astype(bf16),
            "wout": w_o.astype(bf16),
            "bvec": (b_out[None, :] if c % 4 == 0 else zeros_b).astype(bf16),
            "freqs": rotary_pos_emb,
            "identD": ident,
            "rmatD": rmatT.astype(bf16),
            "identB": identb,
        })
    return in_maps


def kernel(x, rotary_pos_emb, w_qkv, w_out, b_out):
    from concourse.bass_utils import run_bass_kernel_spmd

    nc = _get_program()
    in_maps = make_in_maps(x, rotary_pos_emb, w_qkv, w_out, b_out)
    res = run_bass_kernel_spmd(nc, in_maps, list(range(NCORES))).results

    out = np.zeros((B, N, DIM), np.float32)
    for c in range(NCORES):
        out[c // 4] += res[c]["out"]
    return out
